# revision 48
# baseline (speedup 1.0000x reference)
"""Trainium2 Bass kernel for nn_AttCM: 1x1-conv stem -> (two 3x3 convs) +
(single-head spatial attention), alpha/beta combined.

Sharding: 8 cores = 4 samples x 2 halves of the attention key axis (n).
Each core computes the full stem + q for its sample (cheap), its n-half of
S = k^T q with full softmax rows (softmax axis is m, fully local), a partial
attn_out = (v/l) @ exp(S) (host adds the two partials), and half of the 3x3
conv branch rows. No cross-core communication; the host applies
alpha*conv + beta*attn and the inverse of the per-core pixel roll.

SPMD trick: all 8 cores run one graph. Per-core behavior comes from data:
  - xq is the sample pixel-rolled by -2048*h so the core's k/v half is always
    columns [0, 2048) of its local x3; the attention output columns are rolled
    back on the host.
  - xc is a 36-row window of the sample (host zero-padded at image borders)
    so the conv branch always computes local output rows 2..33.
  - mtop/mbot (0.0 or 1.0 per core) zero the stem-of-zero padding rows that
    a true conv 'SAME' zero-pad requires.

Precision: matmul inputs are bf16 (fp32 PSUM accumulation), except k/q which
are fp8-e4m3 scaled x64 so S = k^T q runs as fp8 DoubleRow matmuls (K=256 per
instruction, ~1.5x TensorE throughput); the x4096 scale is compensated for
free inside the ACT exp (scale=1/4096). The attention output matmul also runs
fp8 DoubleRow: the softmax here is near-uniform (S in ~[-0.33, 0.30]), so
attn = u + v @ dev where u[c] = sum_n v[c,n]/4096 is computed exactly in bf16
(K=1-col matmuls, folded into the psum evacuation) and only the deviation
dev = (exp(S)/l - 1/4096)*2^G goes through fp8e4 (G=16 centers it in e4m3
range; plain fp8 softmax would flush the ~2.4e-4 rows to zero). Simulated
rel_l2 vs the fp32 reference: 2.42e-3 (same as all-bf16).

Schedule notes: the S loop is ScalarE-bound (exp of 8.4M elements/core, with
a per-instruction READ_ACCUMULATOR for the softmax row sums), so the 3x3
conv matmuls are woven between S blocks in single-psum-bank pieces to keep
TensorE busy while ScalarE drains exp. PSUM runs as 4 slots of 2 banks.
1/l is folded into vT progressively after each S block so the attention
output matmuls start immediately after the last block. Evacuations alternate
between ScalarE and VectorE to balance engine load. Measured ~211us on
silicon at full clock (incl. ~17us fixed preamble/drain overhead; the
shared device sometimes throttles the PE to ~2.0GHz, measuring ~252us).
"""

import numpy as np
import ml_dtypes

_CACHE = {}

B, C, H, W = 4, 256, 64, 64
N = H * W            # 4096 pixels
NH = N // 2          # per-core attention key half
NB = 16              # n-blocks of 128 rows per core
G = 16               # 2^G boost of the softmax deviation before fp8e4 cast


def _build_nc():
    from contextlib import ExitStack

    import concourse.mybir as mybir
    import concourse.tile as tile
    from concourse import bacc

    f32 = mybir.dt.float32
    bf16 = mybir.dt.bfloat16
    f8 = mybir.dt.float8e4
    AF = mybir.ActivationFunctionType
    AX = mybir.AxisListType

    nc = bacc.Bacc("TRN2", target_bir_lowering=False, debug=False)

    def din(name, shape, dt=bf16):
        return nc.dram_tensor(name, shape, dt, kind="ExternalInput").ap()

    xq_d = din("xq", [3, N])
    wsb2_d = din("wsb2", [128, 466])
    wvb_d = din("wvb", [128, 768])
    wqk8_d = din("wqk8", [128, 1024], f8)
    wb1_d = din("wb1", [128, 2, 9, 256])
    wb2_d = din("wb2", [128, 2, 9, 256])

    oa_d = nc.dram_tensor("out_attn", [C, N], bf16, kind="ExternalOutput").ap()
    oc_d = nc.dram_tensor("out_conv", [C, 32 * 64], bf16, kind="ExternalOutput").ap()

    with tile.TileContext(nc) as tc, ExitStack() as ctx:
        singles = ctx.enter_context(tc.tile_pool(name="singles", bufs=1))
        # two independent 2-buf psum pools (4 banks each): S-blocks/stem on
        # `ps`, conv pieces + early attn groups on `psc`, so attention
        # accumulators can stay open across s_blocks 14/15 without a
        # round-robin cycle against the S chunks.
        ps = ctx.enter_context(tc.tile_pool(name="ps", bufs=2, space="PSUM"))
        psc = ctx.enter_context(tc.tile_pool(name="psc", bufs=2, space="PSUM"))
        big = ctx.enter_context(tc.tile_pool(name="big", bufs=1))

        def load(d, shape, dt=bf16, tag=None):
            nm = d.tensor.name + "_sb"
            t = (singles.tile(shape, dt, tag=tag, name=nm) if tag
                 else singles.tile(shape, dt, name=nm))
            nc.sync.dma_start(out=t, in_=d)
            return t

        xq = big.tile([3, N], bf16, tag="x_in")
        nc.sync.dma_start(out=xq, in_=xq_d)  # first on sync queue: gates h1
        wsb2 = singles.tile([128, 466], bf16, name="wsb2")
        wvb = singles.tile([128, 768], bf16, name="wvb")
        wqk8 = singles.tile([128, 1024], f8, name="wqk8")
        # stem weights + biases split over the 3 DMA-capable queues: one queue
        # serializes 128 row-descriptors (~37 ns each), delaying the stem.
        # w1t (rows 0:3) ships first so h1 can start as soon as xq lands.
        nc.sync.dma_start(out=wsb2[0:3, :], in_=wsb2_d[0:3, :])
        nc.sync.dma_start(out=wsb2[3:44, :], in_=wsb2_d[3:44, :])
        nc.scalar.dma_start(out=wsb2[44:88, :], in_=wsb2_d[44:88, :])
        nc.gpsimd.dma_start(out=wsb2[88:128, :], in_=wsb2_d[88:128, :])
        # bulk weights (wqk8/wvb/wb1/wb2) kick from the scalar queue *after*
        # stem work starts: their ~500 descriptors otherwise contend with the
        # small early loads on the shared DMA channels and delay the biases.
        w1t = wsb2[0:3, 0:64]
        w2t = wsb2[0:64, 64:192]
        w3t = wsb2[:, 192:448]
        wvt = wvb[:, 0:512].rearrange("p (a b) -> p a b", a=2)
        bv = wvb[0:1, 512:768]
        # biases ship as bf16 inside wsb2 (a separate narrow f32 tensor costs
        # 128 DMA descriptors); DVE scalar operands must be f32, so convert
        # the 18 bias columns on-chip once.
        fsb = singles.tile([128, 18], f32, name="fsb32")
        nc.vector.tensor_copy(fsb, wsb2[:, 448:466])
        b1 = fsb[0:64, 0:1]
        b2 = fsb[:, 1:2]
        b3 = fsb[:, 2:4]
        bb1 = fsb[:, 4:6]
        bb2 = fsb[:, 6:8]
        mtop = fsb[:, 8:9]
        mbot = fsb[:, 9:10]
        bq64 = fsb[:, 10:12]
        bk64 = fsb[:, 12:14]
        bq256 = fsb[:, 14:16]
        bk256 = fsb[:, 16:18]
        wq8t = wqk8[:, 0:512].rearrange("p (a b) -> p a b", a=2)
        wk8t = wqk8[:, 512:1024].rearrange("p (a b) -> p a b", a=2)
        ones = singles.tile([1, 128], bf16)
        nc.vector.memset(ones, 1.0)
        ones128 = singles.tile([128, 1], bf16)
        nc.vector.memset(ones128, 1.0)
        lall = singles.tile([128, NB], f32)
        lg = singles.tile([128, NB], f32)
        rl = singles.tile([128, NB], f32)
        u2G = singles.tile([128, 2], f32)
        u1 = singles.tile([128, 2], f32)

        # ---- stem on the rolled full sample (feeds q, k, v) ----
        # evacuations run 512-col halves on ScalarE and VectorE in parallel:
        # the stem is a latency chain (h1 -> h2 -> x3), so evac latency, not
        # throughput, sets its span.
        def pstile(i, part=128, name="p_st"):
            # stem-phase chunks alternate between the two psum pools so the
            # evac-latency chain overlaps 4 tiles even with 2-buf pools
            pool, tg = (ps, "ps") if i % 2 == 0 else (psc, "psc")
            return pool.tile([part, 1024], f32, tag=tg, name=name)

        def relu_evac(dst, p, b, t):
            sh = t % 2
            vh = 1 - sh
            nc.scalar.activation(dst[:, sh * 512 : sh * 512 + 512],
                                 p[:, sh * 512 : sh * 512 + 512], AF.Relu, bias=b)
            nc.vector.tensor_scalar(dst[:, vh * 512 : vh * 512 + 512],
                                    p[:, vh * 512 : vh * 512 + 512], b, 0.0,
                                    op0=mybir.AluOpType.add, op1=mybir.AluOpType.max)

        h1 = big.tile([64, N], bf16, tag="h1")
        for t in range(4):
            p = pstile(t, part=64, name="p_h1")
            for su in range(2):
                nc.tensor.matmul(
                    p[:, su * 512 : (su + 1) * 512], w1t,
                    xq[:, t * 1024 + su * 512 : t * 1024 + (su + 1) * 512],
                    start=True, stop=True,
                )
            relu_evac(h1[:, t * 1024 : (t + 1) * 1024], p, b1, t)
        nc.scalar.dma_start(out=wqk8, in_=wqk8_d)
        nc.scalar.dma_start(out=wvb, in_=wvb_d)
        h2 = big.tile([128, N], bf16, tag="h2")
        for t in range(4):
            p = pstile(t + 1, name="p_h2")
            for su in range(2):
                nc.tensor.matmul(
                    p[:, su * 512 : (su + 1) * 512], w2t,
                    h1[:, t * 1024 + su * 512 : t * 1024 + (su + 1) * 512],
                    start=True, stop=True,
                )
            relu_evac(h2[:, t * 1024 : (t + 1) * 1024], p, b2, t)
        # x3 runs t-major so both cc chunks of an m-range finish together and
        # the q projection (which needs both ki of a chunk) can start early;
        # the fp8 x38 chunk-casts alternate ScalarE/VectorE right behind.
        x3q = big.tile([128, 2, N], bf16, tag="x3q")
        x38 = big.tile([128, 2, N], f8, tag="h1")  # reuses dead h1 space
        for t in range(4):
            for cc in range(2):
                p = pstile(t * 2 + cc, name="p_x3q")
                for su in range(2):
                    nc.tensor.matmul(
                        p[:, su * 512 : (su + 1) * 512],
                        w3t[:, cc * 128 : (cc + 1) * 128],
                        h2[:, t * 1024 + su * 512 : t * 1024 + (su + 1) * 512],
                        start=True, stop=True,
                    )
                csl = slice(t * 1024, (t + 1) * 1024)
                relu_evac(x3q[:, cc, csl], p, b3[:, cc : cc + 1], t + cc)
                if (t + cc) % 2:
                    nc.scalar.activation(x38[:, cc, csl], x3q[:, cc, csl],
                                         AF.Identity, scale=8.0)
                else:
                    nc.vector.tensor_scalar_mul(x38[:, cc, csl], x3q[:, cc, csl], 8.0)

        # ---- k, q (t-major: both cc of a chunk land together so the S loop
        #      can start after chunk 0), fp8 DoubleRow projections from x38 ----
        k_ = big.tile([128, 2, NH], f8, tag="k")
        for t in range(2):
            for cc in range(2):
                p = pstile(t * 2 + cc, name="p_k")
                for su in range(2):
                    nc.tensor.matmul(
                        p[:, su * 512 : (su + 1) * 512],
                        wk8t[:, :, cc * 128 : (cc + 1) * 128],
                        x38[:, :, t * 1024 + su * 512 : t * 1024 + (su + 1) * 512],
                        start=True, stop=True,
                        perf_mode=mybir.MatmulPerfMode.DoubleRow,
                    )
                sh = (t + cc) % 2
                vh = 1 - sh
                nc.scalar.activation(
                    k_[:, cc, t * 1024 + sh * 512 : t * 1024 + sh * 512 + 512],
                    p[:, sh * 512 : sh * 512 + 512], AF.Identity,
                    bias=bk64[:, cc : cc + 1], scale=0.25,
                )
                nc.vector.tensor_scalar(
                    k_[:, cc, t * 1024 + vh * 512 : t * 1024 + vh * 512 + 512],
                    p[:, vh * 512 : vh * 512 + 512], bk256[:, cc : cc + 1], 0.25,
                    op0=mybir.AluOpType.add, op1=mybir.AluOpType.mult,
                )
        q = big.tile([128, 2, N], f8, tag="q")
        for t in range(4):
            for cc in range(2):
                p = pstile(t * 2 + cc, name="p_q")
                for su in range(2):
                    nc.tensor.matmul(
                        p[:, su * 512 : (su + 1) * 512],
                        wq8t[:, :, cc * 128 : (cc + 1) * 128],
                        x38[:, :, t * 1024 + su * 512 : t * 1024 + (su + 1) * 512],
                        start=True, stop=True,
                        perf_mode=mybir.MatmulPerfMode.DoubleRow,
                    )
                sh = (t + cc) % 2
                vh = 1 - sh
                nc.scalar.activation(
                    q[:, cc, t * 1024 + sh * 512 : t * 1024 + sh * 512 + 512],
                    p[:, sh * 512 : sh * 512 + 512], AF.Identity,
                    bias=bq64[:, cc : cc + 1], scale=0.25,
                )
                nc.vector.tensor_scalar(
                    q[:, cc, t * 1024 + vh * 512 : t * 1024 + vh * 512 + 512],
                    p[:, vh * 512 : vh * 512 + 512], bq256[:, cc : cc + 1], 0.25,
                    op0=mybir.AluOpType.add, op1=mybir.AluOpType.mult,
                )

        # vT[n, c] = sum_ci x3[ci, n] WvT[ci, c] + bv[c]  (bias via K=1 matmul)
        # issued later, woven into s_blocks 0/1 to fill TensorE while the
        # Scalar/Vector queues drain the stem evac backlog.
        vT = big.tile([128, NB, 256], bf16, tag="vT")
        vT8 = big.tile([128, NB, 256], f8, tag="vT8")

        def vt_group(g):
            p = pstile(g, name="p_vT")
            for j in range(4):
                nb = g * 4 + j
                nsl = slice(nb * 128, (nb + 1) * 128)
                o = slice(j * 256, (j + 1) * 256)
                nc.tensor.matmul(p[:, o], x3q[:, 0, nsl], wvt[:, 0, :], start=True, stop=False)
                nc.tensor.matmul(p[:, o], x3q[:, 1, nsl], wvt[:, 1, :], start=False, stop=False)
                nc.tensor.matmul(p[:, o], ones, bv, start=False, stop=True)
            nc.vector.tensor_copy(vT[:, g * 4 : (g + 1) * 4, :], p)
            nc.vector.tensor_copy(vT8[:, g * 4 : (g + 1) * 4, :], vT[:, g * 4 : (g + 1) * 4, :])

        # mean term u[c] = (sum_n v[c, n]) / 4096, exact in bf16; folded into
        # the attention-psum evacuation. u2G = u*2^G for the VectorE evac
        # (psum + u2G)*2^-G; u1 = u for the ScalarE evac 2^-G*psum + u.
        def u_mms():
            p_u = pstile(1, name="p_u")
            for cc in range(2):
                for nb in range(NB):
                    nc.tensor.matmul(
                        p_u[:, cc : cc + 1], vT[:, nb, cc * 128 : (cc + 1) * 128],
                        ones128, start=(nb == 0), stop=(nb == NB - 1),
                    )
            nc.scalar.activation(u2G, p_u[:, 0:2], AF.Identity, scale=float(2.0 ** G) / 4096.0)
            nc.scalar.activation(u1, p_u[:, 0:2], AF.Identity, scale=1.0 / 4096.0)

        # ---- conv input: x3c is x3q in the rolled frame — local window row
        #      j (0..35) = rolled row (j-2) mod 64; the per-core mtop/mbot
        #      masks zero the rows that are conv 'SAME' padding (the wrap rows
        #      land exactly where the masks already zero or keep correctly).
        #      cc=0 builds on the otherwise-idle GpSimd, cc=1 on Vector/Scalar.
        x3c = big.tile([128, 2, 36, 66], bf16, tag="x3c")
        # only cols 0 and 65 are never written below — zero just the borders
        nc.gpsimd.memset(x3c[:, :, :, 0:1], 0.0)
        nc.gpsimd.memset(x3c[:, :, :, 65:66], 0.0)
        for cc in range(2):
            nc.vector.tensor_copy(
                x3c[:, cc, 2:36, 1:65],
                x3q[:, cc, 0 : 34 * 64].rearrange("p (a b) -> p a b", a=34),
            )
            nc.vector.tensor_copy(
                x3c[:, cc, 0:2, 1:65],
                x3q[:, cc, 62 * 64 : 64 * 64].rearrange("p (a b) -> p a b", a=2),
            )
        # zero the stem-of-zero border rows (true 'SAME' pad is zero in x3)
        for cc in range(2):
            nc.vector.tensor_scalar_mul(x3c[:, cc, 0:2, :], x3c[:, cc, 0:2, :], mtop)
            nc.vector.tensor_scalar_mul(x3c[:, cc, 34:36, :], x3c[:, cc, 34:36, :], mbot)

        wb1 = singles.tile([128, 2, 9, 256], bf16, tag="wb", name="wb1_sb")
        nc.scalar.dma_start(out=wb1, in_=wb1_d)
        wb2 = singles.tile([128, 2, 9, 256], bf16, tag="wb2", name="wb2_sb")
        nc.scalar.dma_start(out=wb2, in_=wb2_d)
        y1p0 = big.tile([128, 34, 66], bf16, tag="h1")
        y1p1 = big.tile([128, 34, 66], bf16, tag="x_in")
        y1p_ = lambda ki: y1p0 if ki == 0 else y1p1
        # conv1 evacs fill cols 1:65 of every row; zero only the borders
        for yp in (y1p0, y1p1):
            nc.gpsimd.memset(yp[:, :, 0:1], 0.0)
            nc.gpsimd.memset(yp[:, :, 65:66], 0.0)

        # ---- S-loop / conv pieces (interleaved below) ----
        # P8[n, m] = (exp(S)/l - 1/4096) * 2^G in fp8e4: the softmax here is
        # near-uniform, so only the *deviation* from the uniform 1/4096 row
        # goes through fp8 (the exact mean term u is added at evacuation).
        P8 = big.tile([128, NB, N], f8, tag="P8")

        def s_block(nb, weave=()):
            nsl = slice(nb * 128, (nb + 1) * 128)
            lp = singles.tile([128, 4], f32, tag="lp", bufs=4, name="lp")
            pst = big.tile([128, N], bf16, tag="Pst", bufs=3, name="Pst")
            for t in range(4):
                p = ps.tile([128, 1024], f32, tag="ps", name="p_s")
                for su in range(2):
                    o = t * 1024 + su * 512
                    nc.tensor.matmul(
                        p[:, su * 512 : (su + 1) * 512],
                        k_[:, :, nsl], q[:, :, o : o + 512],
                        start=True, stop=True,
                        perf_mode=mybir.MatmulPerfMode.DoubleRow,
                    )
                if t < len(weave) and weave[t] is not None:
                    weave[t]()
                nc.scalar.activation(
                    pst[:, t * 1024 : (t + 1) * 1024], p, AF.Exp,
                    scale=1.0 / 4096.0, accum_out=lp[:, t : t + 1],
                )
            nc.vector.reduce_sum(out=lall[:, nb : nb + 1], in_=lp, axis=AX.X)
            nc.vector.tensor_scalar_mul(lg[:, nb : nb + 1], lall[:, nb : nb + 1],
                                        float(2.0 ** -G))
            nc.vector.reciprocal(rl[:, nb : nb + 1], lg[:, nb : nb + 1])
            # the last two blocks cast in m-halves: the j=7 attention matmuls
            # of the early groups (t<2) unlock after the first half
            halves = 2 if nb >= NB - 2 else 1
            for hh in range(halves):
                msl = slice(hh * (N // halves), (hh + 1) * (N // halves))
                nc.vector.tensor_scalar(
                    P8[:, nb, msl], pst[:, msl], rl[:, nb : nb + 1],
                    -(2.0 ** G) / 4096.0,
                    op0=mybir.AluOpType.mult, op1=mybir.AluOpType.add,
                )

        def conv1_piece(cc, y1row0, nr=8):
            """nr y1-rows in one psum bank."""
            w = nr * 64
            p = psc.tile([128, 1024], f32, tag="psc", name="p_c1")
            for kt in range(18):
                ki, tap = kt // 9, kt % 9
                dh, dw = tap // 3, tap % 3
                nc.tensor.matmul(
                    p[:, 0:w],
                    wb1[:, ki, tap, cc * 128 : (cc + 1) * 128],
                    x3c[:, ki, y1row0 - 1 + dh : y1row0 - 1 + dh + nr, dw : dw + 64],
                    start=(kt == 0), stop=(kt == 17),
                )
            nc.vector.tensor_scalar(
                y1p_(cc)[:, y1row0 - 1 : y1row0 - 1 + nr, 1:65], p[:, 0:w],
                bb1[:, cc : cc + 1], 0.0,
                op0=mybir.AluOpType.add, op1=mybir.AluOpType.max,
            )

        def conv2_piece(cc, orow0, wb2, sti, nr=8):
            w = nr * 64
            p = psc.tile([128, 1024], f32, tag="psc", name="p_c2")
            for kt in range(18):
                ki, tap = kt // 9, kt % 9
                dh, dw = tap // 3, tap % 3
                nc.tensor.matmul(
                    p[:, 0:w],
                    wb2[:, ki, tap, cc * 128 : (cc + 1) * 128],
                    y1p_(ki)[:, orow0 - 2 + dh : orow0 - 2 + dh + nr, dw : dw + 64],
                    start=(kt == 0), stop=(kt == 17),
                )
            st = big.tile([128, 1024], bf16, tag=("h2" if sti else "x3c"), name="st_c")
            nc.vector.tensor_scalar_add(st[:, 0:w], p[:, 0:w], bb2[:, cc : cc + 1])
            (nc.sync if sti else nc.gpsimd).dma_start(
                out=oc_d[cc * 128 : (cc + 1) * 128, (orow0 - 2) * 64 : (orow0 - 2) * 64 + w],
                in_=st[:, 0:w],
            )

        # ---- interleave: S blocks are ScalarE(exp)-paced; vT/u then conv
        #      groups keep TensorE busy meanwhile ----
        s_block(0, weave=[lambda: vt_group(0), lambda: vt_group(1),
                          lambda: vt_group(2), lambda: vt_group(3)])
        s_block(1, weave=[u_mms, None, lambda: conv1_piece(0, 1)])
        conv1_piece(0, 9)
        s_block(2)
        conv1_piece(0, 17)
        conv1_piece(0, 25)
        s_block(3)
        conv1_piece(1, 1)
        conv1_piece(1, 9)
        s_block(4)
        conv1_piece(1, 17)
        conv1_piece(1, 25)
        s_block(5)
        conv1_piece(0, 33, nr=2)
        conv1_piece(1, 33, nr=2)
        for cc in range(2):
            nc.vector.tensor_scalar_mul(y1p_(cc)[:, 0, :], y1p_(cc)[:, 0, :], mtop)
            nc.vector.tensor_scalar_mul(y1p_(cc)[:, 33, :], y1p_(cc)[:, 33, :], mbot)
        s_block(6)
        conv2_piece(0, 2, wb2, 0)
        s_block(7)
        conv2_piece(0, 10, wb2, 1)
        s_block(8)
        conv2_piece(0, 18, wb2, 0)
        s_block(9)
        conv2_piece(0, 26, wb2, 1)
        s_block(10)
        conv2_piece(1, 2, wb2, 0)
        s_block(11)
        conv2_piece(1, 10, wb2, 1)
        # ---- attn_out partial = v @ P8 * 2^-G + u; fp8 DoubleRow (K=256 per
        #      instruction over nb pairs). Groups 0/1 open early and weave
        #      their j<=6 pairs (which only need P8 blocks <=13) into
        #      s_blocks 14/15, where no conv work is left to fill TensorE
        #      while ScalarE drains the last exps. ----
        attn_ps = {}

        def attn_pairs(uu, pairs):
            cc, t = uu // 4, uu % 4
            if uu not in attn_ps:
                pool = psc if uu < 2 or uu % 2 == 0 else ps
                attn_ps[uu] = pool.tile([128, 1024], f32,
                                        tag=("psc" if pool is psc else "ps"),
                                        name="p_at")
            p = attn_ps[uu]
            for j in pairs:
                for su in range(2):
                    o = t * 1024 + su * 512
                    nc.tensor.matmul(
                        p[:, su * 512 : (su + 1) * 512],
                        vT8[:, 2 * j : 2 * j + 2, cc * 128 : (cc + 1) * 128],
                        P8[:, 2 * j : 2 * j + 2, o : o + 512],
                        start=(j == 0), stop=(j == NB // 2 - 1),
                        perf_mode=mybir.MatmulPerfMode.DoubleRow,
                    )

        def attn_close(uu):
            cc, t = uu // 4, uu % 4
            p = attn_ps.pop(uu)
            st = big.tile([128, 1024], bf16, tag=("h2" if uu % 2 else "x3c"), name="st_a")
            nc.vector.tensor_scalar(
                st[:, 0:512], p[:, 0:512], u2G[:, cc : cc + 1], float(2.0 ** -G),
                op0=mybir.AluOpType.add, op1=mybir.AluOpType.mult,
            )
            nc.scalar.activation(
                st[:, 512:1024], p[:, 512:1024], AF.Identity,
                bias=u1[:, cc : cc + 1], scale=float(2.0 ** -G),
            )
            (nc.sync if uu % 2 else nc.gpsimd).dma_start(
                out=oa_d[cc * 128 : (cc + 1) * 128, t * 1024 : t * 1024 + 512],
                in_=st[:, 0:512],
            )
            (nc.scalar if uu % 2 else nc.sync).dma_start(
                out=oa_d[cc * 128 : (cc + 1) * 128, t * 1024 + 512 : (t + 1) * 1024],
                in_=st[:, 512:1024],
            )

        s_block(12)
        conv2_piece(1, 18, wb2, 0)
        s_block(13)
        conv2_piece(1, 26, wb2, 1)
        s_block(14, weave=[
            lambda: attn_pairs(0, [0, 1]), lambda: attn_pairs(0, [2, 3]),
            lambda: attn_pairs(0, [4, 5]), lambda: attn_pairs(0, [6]),
        ])
        s_block(15, weave=[
            lambda: attn_pairs(1, [0, 1]), lambda: attn_pairs(1, [2, 3]),
            lambda: attn_pairs(1, [4, 5]), lambda: attn_pairs(1, [6]),
        ])
        attn_pairs(0, [7])
        attn_close(0)
        attn_pairs(1, [7])
        attn_close(1)
        # remaining groups run pairwise with the j-loop outer (the pair shares
        # cc so one LDWEIGHTS serves both); closes are software-pipelined
        # behind the next pair's first matmuls to hide evac latency.
        for j in range(NB // 2):
            attn_pairs(2, [j])
            attn_pairs(3, [j])
        for j in range(2):
            attn_pairs(4, [j])
            attn_pairs(5, [j])
        attn_close(2)
        attn_close(3)
        for j in range(2, NB // 2):
            attn_pairs(4, [j])
            attn_pairs(5, [j])
        for j in range(2):
            attn_pairs(6, [j])
            attn_pairs(7, [j])
        attn_close(4)
        attn_close(5)
        for j in range(2, NB // 2):
            attn_pairs(6, [j])
            attn_pairs(7, [j])
        attn_close(6)
        attn_close(7)

    nc.compile()
    return nc


def _get_nc():
    if "nc" not in _CACHE:
        _CACHE["nc"] = _build_nc()
    return _CACHE["nc"]


def _make_in_maps(x, w1, b1, w2, b2, w3, b3, wb1, bb1, wb2, bb2,
                  wq, bq, wk, bk, wv, bv):
    bfc = lambda a: np.ascontiguousarray(np.asarray(a, np.float32).astype(ml_dtypes.bfloat16))
    f32c = lambda a: np.ascontiguousarray(np.asarray(a, np.float32))

    def qkv_t(w):  # [O, CI] -> lhsT/rhs chunks [128, 2, 256]
        return bfc(np.asarray(w, np.float32).T.reshape(2, 128, 256).transpose(1, 0, 2))

    def conv_t(wb):  # [O, I, 3, 3] -> [128 kip, 2 ki, 9 tap, 256 o]
        a = np.asarray(wb, np.float32).transpose(1, 0, 2, 3)  # [I, O, 3, 3]
        a = a.reshape(2, 128, 256, 9)                          # [ki, kip, o, tap]
        return bfc(a.transpose(1, 0, 3, 2))                    # [kip, ki, tap, o]

    def bias2(b):  # [256] -> [128, 2] (col cc = chunk cc)
        return f32c(np.asarray(b, np.float32).reshape(2, 128).T)

    wsb2 = np.zeros((128, 466), np.float32)
    wsb2[0:3, 0:64] = np.asarray(w1).T
    wsb2[0:64, 64:192] = np.asarray(w2).T
    wsb2[:, 192:448] = np.asarray(w3).T
    wsb2[0:64, 448] = np.asarray(b1)
    wsb2[:, 449] = np.asarray(b2)
    wsb2[:, 450:452] = bias2(b3)
    wsb2[:, 452:454] = bias2(bb1)
    wsb2[:, 454:456] = bias2(bb2)
    # cols 456/457 = per-core mtop/mbot, filled below
    wsb2[:, 458:460] = bias2(bq) * 64.0
    wsb2[:, 460:462] = bias2(bk) * 64.0
    wsb2[:, 462:464] = bias2(bq) * 256.0
    wsb2[:, 464:466] = bias2(bk) * 256.0
    wvb = np.zeros((128, 768), np.float32)
    wvb[:, 0:512] = qkv_t(wv).astype(np.float32).reshape(128, 512)
    wvb[0, 512:768] = np.asarray(bv)
    wqk8 = np.zeros((128, 1024), np.float32)
    wqk8[:, 0:512] = qkv_t(wq).astype(np.float32).reshape(128, 512) * 32.0
    wqk8[:, 512:1024] = qkv_t(wk).astype(np.float32).reshape(128, 512) * 32.0
    common = {
        "wvb": bfc(wvb),
        "wqk8": np.ascontiguousarray(wqk8.astype(ml_dtypes.float8_e4m3)),
        "wb1": conv_t(wb1),
        "wb2": conv_t(wb2),
    }

    xf = np.asarray(x, np.float32).reshape(B, 3, N)
    in_maps = []
    for core in range(8):
        b, h = core // 2, core % 2
        xq = bfc(np.roll(xf[b], -NH * h, axis=1))
        # conv window: global rows [32h-2, 32h+34), zero outside the image
        wc = wsb2.copy()
        wc[:, 456] = 0.0 if h == 0 else 1.0
        wc[:, 457] = 1.0 if h == 0 else 0.0
        in_maps.append(dict(
            common,
            xq=xq,
            wsb2=bfc(wc),
        ))
    return in_maps


def _gather(results, alpha, beta):
    a, bt = float(alpha), float(beta)
    out = np.empty((B, C, H, W), np.float32)
    for b in range(B):
        r0, r1 = results[2 * b], results[2 * b + 1]
        oa0 = np.asarray(r0["out_attn"], np.float32)
        oa1 = np.asarray(r1["out_attn"], np.float32)
        attn = oa0 + np.roll(oa1, NH, axis=1)
        conv = np.concatenate(
            [np.asarray(r0["out_conv"], np.float32).reshape(C, 32, W),
             np.asarray(r1["out_conv"], np.float32).reshape(C, 32, W)],
            axis=1,
        )
        out[b] = a * conv + bt * attn.reshape(C, H, W)
    return out


def _run(inputs, trace=False, **kw):
    from concourse import bass_utils

    nc = _get_nc()
    in_maps = _make_in_maps(
        inputs["x"], inputs["w1"], inputs["b1"], inputs["w2"], inputs["b2"],
        inputs["w3"], inputs["b3"], inputs["wb1"], inputs["bb1"],
        inputs["wb2"], inputs["bb2"], inputs["wq"], inputs["bq"],
        inputs["wk"], inputs["bk"], inputs["wv"], inputs["bv"],
    )
    res = bass_utils.run_bass_kernel_spmd(
        nc, in_maps, core_ids=list(range(8)), trace=trace, **kw
    )
    return _gather(res.results, inputs["alpha"], inputs["beta"]), res


def kernel(**inputs):
    out, _ = _run(inputs, trace=False)
    return out



# revision 55
# speedup vs baseline: 1.0072x; 1.0072x over previous
"""Trainium2 Bass kernel for nn_AttCM: 1x1-conv stem -> (two 3x3 convs) +
(single-head spatial attention), alpha/beta combined.

Sharding: 8 cores = 4 samples x 2 halves of the attention key axis (n).
Each core computes the full stem + q for its sample (cheap), its n-half of
S = k^T q with full softmax rows (softmax axis is m, fully local), a partial
attn_out = (v/l) @ exp(S) (host adds the two partials), and half of the 3x3
conv branch rows. No cross-core communication; the host applies
alpha*conv + beta*attn and the inverse of the per-core pixel roll.

SPMD trick: all 8 cores run one graph. Per-core behavior comes from data:
  - xq is the sample pixel-rolled by -2048*h so the core's k/v half is always
    columns [0, 2048) of its local x3; the attention output columns are rolled
    back on the host.
  - xc is a 36-row window of the sample (host zero-padded at image borders)
    so the conv branch always computes local output rows 2..33.
  - mtop/mbot (0.0 or 1.0 per core) zero the stem-of-zero padding rows that
    a true conv 'SAME' zero-pad requires.

Precision: matmul inputs are bf16 (fp32 PSUM accumulation), except k/q which
are fp8-e4m3 scaled x64 so S = k^T q runs as fp8 DoubleRow matmuls (K=256 per
instruction, ~1.5x TensorE throughput); the x4096 scale is compensated for
free inside the ACT exp (scale=1/4096). The attention output matmul also runs
fp8 DoubleRow: the softmax here is near-uniform (S in ~[-0.33, 0.30]), so
attn = u + v @ dev where u[c] = sum_n v[c,n]/4096 is computed exactly in bf16
(K=1-col matmuls, folded into the psum evacuation) and only the deviation
dev = (exp(S)/l - 1/4096)*2^G goes through fp8e4 (G=16 centers it in e4m3
range; plain fp8 softmax would flush the ~2.4e-4 rows to zero). Simulated
rel_l2 vs the fp32 reference: 2.42e-3 (same as all-bf16).

Schedule notes: the S loop is ScalarE-bound (exp of 8.4M elements/core, with
a per-instruction READ_ACCUMULATOR for the softmax row sums), so the 3x3
conv matmuls are woven between S blocks in single-psum-bank pieces to keep
TensorE busy while ScalarE drains exp. PSUM runs as 4 slots of 2 banks.
1/l is folded into vT progressively after each S block so the attention
output matmuls start immediately after the last block. Evacuations alternate
between ScalarE and VectorE to balance engine load. Measured ~211us on
silicon at full clock (incl. ~17us fixed preamble/drain overhead; the
shared device sometimes throttles the PE to ~2.0GHz, measuring ~252us).
"""

import numpy as np
import ml_dtypes

_CACHE = {}

B, C, H, W = 4, 256, 64, 64
N = H * W            # 4096 pixels
NH = N // 2          # per-core attention key half
NB = 16              # n-blocks of 128 rows per core
G = 16               # 2^G boost of the softmax deviation before fp8e4 cast


def _build_nc():
    from contextlib import ExitStack

    import concourse.mybir as mybir
    import concourse.tile as tile
    from concourse import bacc

    f32 = mybir.dt.float32
    bf16 = mybir.dt.bfloat16
    f8 = mybir.dt.float8e4
    AF = mybir.ActivationFunctionType
    AX = mybir.AxisListType

    nc = bacc.Bacc("TRN2", target_bir_lowering=False, debug=False)

    def din(name, shape, dt=bf16):
        return nc.dram_tensor(name, shape, dt, kind="ExternalInput").ap()

    xq_d = din("xq", [4, N])  # row 3 = ones: b1 rides w1t row 3
    wsb2_d = din("wsb2", [128, 466])
    wvb_d = din("wvb", [128, 768])
    wqk8_d = din("wqk8", [128, 1024], f8)
    wb1_d = din("wb1", [128, 2, 9, 256])
    wb2_d = din("wb2", [128, 2, 9, 256])

    oa_d = nc.dram_tensor("out_attn", [C, N], bf16, kind="ExternalOutput").ap()
    oc_d = nc.dram_tensor("out_conv", [C, 32 * 64], bf16, kind="ExternalOutput").ap()

    with tile.TileContext(nc) as tc, ExitStack() as ctx:
        singles = ctx.enter_context(tc.tile_pool(name="singles", bufs=1))
        # two independent 2-buf psum pools (4 banks each): S-blocks/stem on
        # `ps`, conv pieces + early attn groups on `psc`, so attention
        # accumulators can stay open across s_blocks 14/15 without a
        # round-robin cycle against the S chunks.
        ps = ctx.enter_context(tc.tile_pool(name="ps", bufs=2, space="PSUM"))
        psc = ctx.enter_context(tc.tile_pool(name="psc", bufs=2, space="PSUM"))
        big = ctx.enter_context(tc.tile_pool(name="big", bufs=1))

        def load(d, shape, dt=bf16, tag=None):
            nm = d.tensor.name + "_sb"
            t = (singles.tile(shape, dt, tag=tag, name=nm) if tag
                 else singles.tile(shape, dt, name=nm))
            nc.sync.dma_start(out=t, in_=d)
            return t

        xq = big.tile([4, N], bf16, tag="x_in")
        nc.sync.dma_start(out=xq, in_=xq_d)  # first on sync queue: gates h1
        wsb2 = singles.tile([128, 466], bf16, name="wsb2")
        wvb = singles.tile([128, 768], bf16, name="wvb")
        wqk8 = singles.tile([128, 1024], f8, name="wqk8")
        # stem weights + biases split over the 3 DMA-capable queues: one queue
        # serializes 128 row-descriptors (~37 ns each), delaying the stem.
        # w1t (rows 0:3) ships first so h1 can start as soon as xq lands.
        nc.sync.dma_start(out=wsb2[0:4, :], in_=wsb2_d[0:4, :])
        nc.sync.dma_start(out=wsb2[4:44, :], in_=wsb2_d[4:44, :])
        nc.scalar.dma_start(out=wsb2[44:88, :], in_=wsb2_d[44:88, :])
        nc.gpsimd.dma_start(out=wsb2[88:128, :], in_=wsb2_d[88:128, :])
        # bulk weights (wqk8/wvb/wb1/wb2) kick from the scalar queue *after*
        # stem work starts: their ~500 descriptors otherwise contend with the
        # small early loads on the shared DMA channels and delay the biases.
        w1t = wsb2[0:4, 0:64]    # row 3 = b1 (pairs with xq's ones row)
        w2t = wsb2[0:65, 64:192]  # row 64 = b2 (pairs with h1's ones row)
        w3t = wsb2[:, 192:448]
        wvt = wvb[:, 0:512].rearrange("p (a b) -> p a b", a=2)
        bv = wvb[0:1, 512:768]
        # biases ship as bf16 inside wsb2 (a separate narrow f32 tensor costs
        # 128 DMA descriptors); DVE scalar operands must be f32, so convert
        # the 18 bias columns on-chip once.
        fsb = singles.tile([128, 18], f32, name="fsb32")
        nc.vector.tensor_copy(fsb, wsb2[:, 448:466])
        b1 = fsb[0:64, 0:1]
        b2 = fsb[:, 1:2]
        b3 = fsb[:, 2:4]
        bb1 = fsb[:, 4:6]
        bb2 = fsb[:, 6:8]
        mtop = fsb[:, 8:9]
        mbot = fsb[:, 9:10]
        bq64 = fsb[:, 10:12]
        bk64 = fsb[:, 12:14]
        bq256 = fsb[:, 14:16]
        bk256 = fsb[:, 16:18]
        wq8t = wqk8[:, 0:512].rearrange("p (a b) -> p a b", a=2)
        wk8t = wqk8[:, 512:1024].rearrange("p (a b) -> p a b", a=2)
        ones = singles.tile([1, 128], bf16)
        nc.vector.memset(ones, 1.0)
        ones128 = singles.tile([128, 1], bf16)
        nc.vector.memset(ones128, 1.0)
        lall = singles.tile([128, NB], f32)
        lg = singles.tile([128, NB], f32)
        rl = singles.tile([128, NB], f32)
        u2G = singles.tile([128, 2], f32)
        u1 = singles.tile([128, 2], f32)

        # ---- stem on the rolled full sample (feeds q, k, v) ----
        # evacuations run 512-col halves on ScalarE and VectorE in parallel:
        # the stem is a latency chain (h1 -> h2 -> x3), so evac latency, not
        # throughput, sets its span.
        def pstile(i, part=128, name="p_st"):
            # stem-phase chunks alternate between the two psum pools so the
            # evac-latency chain overlaps 4 tiles even with 2-buf pools
            pool, tg = (ps, "ps") if i % 2 == 0 else (psc, "psc")
            return pool.tile([part, 1024], f32, tag=tg, name=name)

        def relu_evac(dst, p, b, t):
            # b=None: bias already folded into the matmul via a ones row
            sh = t % 2
            vh = 1 - sh
            ssl = slice(sh * 512, sh * 512 + 512)
            vsl = slice(vh * 512, vh * 512 + 512)
            if b is None:
                nc.scalar.activation(dst[:, ssl], p[:, ssl], AF.Relu)
                nc.vector.tensor_scalar_max(dst[:, vsl], p[:, vsl], 0.0)
            else:
                nc.scalar.activation(dst[:, ssl], p[:, ssl], AF.Relu, bias=b)
                nc.vector.tensor_scalar(dst[:, vsl], p[:, vsl], b, 0.0,
                                        op0=mybir.AluOpType.add, op1=mybir.AluOpType.max)

        h1 = big.tile([65, N], bf16, tag="h1")
        nc.vector.memset(h1[64:65, :], 1.0)  # ones row: b2 rides w2t row 64
        for t in range(4):
            p = pstile(t, part=64, name="p_h1")
            for su in range(2):
                nc.tensor.matmul(
                    p[:, su * 512 : (su + 1) * 512], w1t,
                    xq[:, t * 1024 + su * 512 : t * 1024 + (su + 1) * 512],
                    start=True, stop=True,
                )
            relu_evac(h1[0:64, t * 1024 : (t + 1) * 1024], p, None, t)
        nc.scalar.dma_start(out=wqk8, in_=wqk8_d)
        nc.scalar.dma_start(out=wvb, in_=wvb_d)
        h2 = big.tile([128, N], bf16, tag="h2")
        for t in range(4):
            p = pstile(t + 1, name="p_h2")
            for su in range(2):
                nc.tensor.matmul(
                    p[:, su * 512 : (su + 1) * 512], w2t,
                    h1[:, t * 1024 + su * 512 : t * 1024 + (su + 1) * 512],
                    start=True, stop=True,
                )
            relu_evac(h2[:, t * 1024 : (t + 1) * 1024], p, None, t)
        # x3 runs t-major so both cc chunks of an m-range finish together and
        # the q projection (which needs both ki of a chunk) can start early;
        # the fp8 x38 chunk-casts alternate ScalarE/VectorE right behind.
        x3q = big.tile([128, 2, N], bf16, tag="x3q")
        x38 = big.tile([128, 2, N], f8, tag="h1")  # reuses dead h1 space
        for t in range(4):
            for cc in range(2):
                p = pstile(t * 2 + cc, name="p_x3q")
                for su in range(2):
                    nc.tensor.matmul(
                        p[:, su * 512 : (su + 1) * 512],
                        w3t[:, cc * 128 : (cc + 1) * 128],
                        h2[:, t * 1024 + su * 512 : t * 1024 + (su + 1) * 512],
                        start=True, stop=True,
                    )
                csl = slice(t * 1024, (t + 1) * 1024)
                relu_evac(x3q[:, cc, csl], p, b3[:, cc : cc + 1], t + cc)
                if (t + cc) % 2:
                    nc.scalar.activation(x38[:, cc, csl], x3q[:, cc, csl],
                                         AF.Identity, scale=8.0)
                else:
                    nc.vector.tensor_scalar_mul(x38[:, cc, csl], x3q[:, cc, csl], 8.0)

        # ---- k, q (t-major: both cc of a chunk land together so the S loop
        #      can start after chunk 0), fp8 DoubleRow projections from x38 ----
        k_ = big.tile([128, 2, NH], f8, tag="k")
        for t in range(2):
            for cc in range(2):
                p = pstile(t * 2 + cc, name="p_k")
                for su in range(2):
                    nc.tensor.matmul(
                        p[:, su * 512 : (su + 1) * 512],
                        wk8t[:, :, cc * 128 : (cc + 1) * 128],
                        x38[:, :, t * 1024 + su * 512 : t * 1024 + (su + 1) * 512],
                        start=True, stop=True,
                        perf_mode=mybir.MatmulPerfMode.DoubleRow,
                    )
                sh = (t + cc) % 2
                vh = 1 - sh
                nc.scalar.activation(
                    k_[:, cc, t * 1024 + sh * 512 : t * 1024 + sh * 512 + 512],
                    p[:, sh * 512 : sh * 512 + 512], AF.Identity,
                    bias=bk64[:, cc : cc + 1], scale=0.25,
                )
                nc.vector.tensor_scalar(
                    k_[:, cc, t * 1024 + vh * 512 : t * 1024 + vh * 512 + 512],
                    p[:, vh * 512 : vh * 512 + 512], bk256[:, cc : cc + 1], 0.25,
                    op0=mybir.AluOpType.add, op1=mybir.AluOpType.mult,
                )
        q = big.tile([128, 2, N], f8, tag="q")
        for t in range(4):
            for cc in range(2):
                p = pstile(t * 2 + cc, name="p_q")
                for su in range(2):
                    nc.tensor.matmul(
                        p[:, su * 512 : (su + 1) * 512],
                        wq8t[:, :, cc * 128 : (cc + 1) * 128],
                        x38[:, :, t * 1024 + su * 512 : t * 1024 + (su + 1) * 512],
                        start=True, stop=True,
                        perf_mode=mybir.MatmulPerfMode.DoubleRow,
                    )
                sh = (t + cc) % 2
                vh = 1 - sh
                nc.scalar.activation(
                    q[:, cc, t * 1024 + sh * 512 : t * 1024 + sh * 512 + 512],
                    p[:, sh * 512 : sh * 512 + 512], AF.Identity,
                    bias=bq64[:, cc : cc + 1], scale=0.25,
                )
                nc.vector.tensor_scalar(
                    q[:, cc, t * 1024 + vh * 512 : t * 1024 + vh * 512 + 512],
                    p[:, vh * 512 : vh * 512 + 512], bq256[:, cc : cc + 1], 0.25,
                    op0=mybir.AluOpType.add, op1=mybir.AluOpType.mult,
                )

        # vT[n, c] = sum_ci x3[ci, n] WvT[ci, c] + bv[c]  (bias via K=1 matmul)
        # issued later, woven into s_blocks 0/1 to fill TensorE while the
        # Scalar/Vector queues drain the stem evac backlog.
        vT = big.tile([128, NB, 256], bf16, tag="vT")
        vT8 = big.tile([128, NB, 256], f8, tag="vT8")

        def vt_group(g):
            p = pstile(g, name="p_vT")
            for j in range(4):
                nb = g * 4 + j
                nsl = slice(nb * 128, (nb + 1) * 128)
                o = slice(j * 256, (j + 1) * 256)
                nc.tensor.matmul(p[:, o], x3q[:, 0, nsl], wvt[:, 0, :], start=True, stop=False)
                nc.tensor.matmul(p[:, o], x3q[:, 1, nsl], wvt[:, 1, :], start=False, stop=False)
                nc.tensor.matmul(p[:, o], ones, bv, start=False, stop=True)
            nc.vector.tensor_copy(vT[:, g * 4 : (g + 1) * 4, :], p)
            nc.vector.tensor_copy(vT8[:, g * 4 : (g + 1) * 4, :], vT[:, g * 4 : (g + 1) * 4, :])

        # mean term u[c] = (sum_n v[c, n]) / 4096, exact in bf16; folded into
        # the attention-psum evacuation. u2G = u*2^G for the VectorE evac
        # (psum + u2G)*2^-G; u1 = u for the ScalarE evac 2^-G*psum + u.
        def u_mms():
            p_u = pstile(1, name="p_u")
            for cc in range(2):
                for nb in range(NB):
                    nc.tensor.matmul(
                        p_u[:, cc : cc + 1], vT[:, nb, cc * 128 : (cc + 1) * 128],
                        ones128, start=(nb == 0), stop=(nb == NB - 1),
                    )
            nc.scalar.activation(u2G, p_u[:, 0:2], AF.Identity, scale=float(2.0 ** G) / 4096.0)
            nc.scalar.activation(u1, p_u[:, 0:2], AF.Identity, scale=1.0 / 4096.0)

        # ---- conv input: x3c is x3q in the rolled frame — local window row
        #      j (0..35) = rolled row (j-2) mod 64; the per-core mtop/mbot
        #      masks zero the rows that are conv 'SAME' padding (the wrap rows
        #      land exactly where the masks already zero or keep correctly).
        #      cc=0 builds on the otherwise-idle GpSimd, cc=1 on Vector/Scalar.
        x3c = big.tile([128, 2, 36, 66], bf16, tag="x3c")
        # only cols 0 and 65 are never written below — zero just the borders
        nc.gpsimd.memset(x3c[:, :, :, 0:1], 0.0)
        nc.gpsimd.memset(x3c[:, :, :, 65:66], 0.0)
        for cc in range(2):
            nc.vector.tensor_copy(
                x3c[:, cc, 2:36, 1:65],
                x3q[:, cc, 0 : 34 * 64].rearrange("p (a b) -> p a b", a=34),
            )
            nc.vector.tensor_copy(
                x3c[:, cc, 0:2, 1:65],
                x3q[:, cc, 62 * 64 : 64 * 64].rearrange("p (a b) -> p a b", a=2),
            )
        # zero the stem-of-zero border rows (true 'SAME' pad is zero in x3)
        for cc in range(2):
            nc.vector.tensor_scalar_mul(x3c[:, cc, 0:2, :], x3c[:, cc, 0:2, :], mtop)
            nc.vector.tensor_scalar_mul(x3c[:, cc, 34:36, :], x3c[:, cc, 34:36, :], mbot)

        wb1 = singles.tile([128, 2, 9, 256], bf16, tag="wb", name="wb1_sb")
        nc.scalar.dma_start(out=wb1, in_=wb1_d)
        wb2 = singles.tile([128, 2, 9, 256], bf16, tag="wb2", name="wb2_sb")
        nc.scalar.dma_start(out=wb2, in_=wb2_d)
        y1p0 = big.tile([128, 34, 66], bf16, tag="h1")
        y1p1 = big.tile([128, 34, 66], bf16, tag="x_in")
        y1p_ = lambda ki: y1p0 if ki == 0 else y1p1
        # conv1 evacs fill cols 1:65 of every row; zero only the borders
        for yp in (y1p0, y1p1):
            nc.gpsimd.memset(yp[:, :, 0:1], 0.0)
            nc.gpsimd.memset(yp[:, :, 65:66], 0.0)

        # ---- S-loop / conv pieces (interleaved below) ----
        # P8[n, m] = (exp(S)/l - 1/4096) * 2^G in fp8e4: the softmax here is
        # near-uniform, so only the *deviation* from the uniform 1/4096 row
        # goes through fp8 (the exact mean term u is added at evacuation).
        P8 = big.tile([128, NB, N], f8, tag="P8")

        def s_block(nb, weave=()):
            nsl = slice(nb * 128, (nb + 1) * 128)
            lp = singles.tile([128, 4], f32, tag="lp", bufs=4, name="lp")
            pst = big.tile([128, N], bf16, tag="Pst", bufs=3, name="Pst")
            for t in range(4):
                p = ps.tile([128, 1024], f32, tag="ps", name="p_s")
                for su in range(2):
                    o = t * 1024 + su * 512
                    nc.tensor.matmul(
                        p[:, su * 512 : (su + 1) * 512],
                        k_[:, :, nsl], q[:, :, o : o + 512],
                        start=True, stop=True,
                        perf_mode=mybir.MatmulPerfMode.DoubleRow,
                    )
                if t < len(weave) and weave[t] is not None:
                    weave[t]()
                nc.scalar.activation(
                    pst[:, t * 1024 : (t + 1) * 1024], p, AF.Exp,
                    scale=1.0 / 4096.0, accum_out=lp[:, t : t + 1],
                )
            nc.vector.reduce_sum(out=lall[:, nb : nb + 1], in_=lp, axis=AX.X)
            nc.vector.tensor_scalar_mul(lg[:, nb : nb + 1], lall[:, nb : nb + 1],
                                        float(2.0 ** -G))
            nc.vector.reciprocal(rl[:, nb : nb + 1], lg[:, nb : nb + 1])
            # the last two blocks cast in m-halves: the j=7 attention matmuls
            # of the early groups (t<2) unlock after the first half
            halves = 2 if nb >= NB - 2 else 1
            for hh in range(halves):
                msl = slice(hh * (N // halves), (hh + 1) * (N // halves))
                nc.vector.tensor_scalar(
                    P8[:, nb, msl], pst[:, msl], rl[:, nb : nb + 1],
                    -(2.0 ** G) / 4096.0,
                    op0=mybir.AluOpType.mult, op1=mybir.AluOpType.add,
                )

        def conv1_piece(cc, y1row0, nr=8):
            """nr y1-rows in one psum bank."""
            w = nr * 64
            p = psc.tile([128, 1024], f32, tag="psc", name="p_c1")
            for kt in range(18):
                ki, tap = kt // 9, kt % 9
                dh, dw = tap // 3, tap % 3
                nc.tensor.matmul(
                    p[:, 0:w],
                    wb1[:, ki, tap, cc * 128 : (cc + 1) * 128],
                    x3c[:, ki, y1row0 - 1 + dh : y1row0 - 1 + dh + nr, dw : dw + 64],
                    start=(kt == 0), stop=(kt == 17),
                )
            nc.vector.tensor_scalar(
                y1p_(cc)[:, y1row0 - 1 : y1row0 - 1 + nr, 1:65], p[:, 0:w],
                bb1[:, cc : cc + 1], 0.0,
                op0=mybir.AluOpType.add, op1=mybir.AluOpType.max,
            )

        def conv2_piece(cc, orow0, wb2, sti, nr=8):
            w = nr * 64
            p = psc.tile([128, 1024], f32, tag="psc", name="p_c2")
            for kt in range(18):
                ki, tap = kt // 9, kt % 9
                dh, dw = tap // 3, tap % 3
                nc.tensor.matmul(
                    p[:, 0:w],
                    wb2[:, ki, tap, cc * 128 : (cc + 1) * 128],
                    y1p_(ki)[:, orow0 - 2 + dh : orow0 - 2 + dh + nr, dw : dw + 64],
                    start=(kt == 0), stop=(kt == 17),
                )
            st = big.tile([128, 1024], bf16, tag=("h2" if sti else "x3c"), name="st_c")
            nc.vector.tensor_scalar_add(st[:, 0:w], p[:, 0:w], bb2[:, cc : cc + 1])
            (nc.sync if sti else nc.gpsimd).dma_start(
                out=oc_d[cc * 128 : (cc + 1) * 128, (orow0 - 2) * 64 : (orow0 - 2) * 64 + w],
                in_=st[:, 0:w],
            )

        # ---- interleave: S blocks are ScalarE(exp)-paced; vT/u then conv
        #      groups keep TensorE busy meanwhile ----
        s_block(0, weave=[lambda: vt_group(0), lambda: vt_group(1),
                          lambda: vt_group(2), lambda: vt_group(3)])
        s_block(1, weave=[u_mms, None, lambda: conv1_piece(0, 1)])
        conv1_piece(0, 9)
        s_block(2)
        conv1_piece(0, 17)
        conv1_piece(0, 25)
        s_block(3)
        conv1_piece(1, 1)
        conv1_piece(1, 9)
        s_block(4)
        conv1_piece(1, 17)
        conv1_piece(1, 25)
        s_block(5)
        conv1_piece(0, 33, nr=2)
        conv1_piece(1, 33, nr=2)
        for cc in range(2):
            nc.vector.tensor_scalar_mul(y1p_(cc)[:, 0, :], y1p_(cc)[:, 0, :], mtop)
            nc.vector.tensor_scalar_mul(y1p_(cc)[:, 33, :], y1p_(cc)[:, 33, :], mbot)
        s_block(6)
        conv2_piece(0, 2, wb2, 0)
        s_block(7)
        conv2_piece(0, 10, wb2, 1)
        s_block(8)
        conv2_piece(0, 18, wb2, 0)
        s_block(9)
        conv2_piece(0, 26, wb2, 1)
        s_block(10)
        conv2_piece(1, 2, wb2, 0)
        s_block(11)
        conv2_piece(1, 10, wb2, 1)
        # ---- attn_out partial = v @ P8 * 2^-G + u; fp8 DoubleRow (K=256 per
        #      instruction over nb pairs). Groups 0/1 open early and weave
        #      their j<=6 pairs (which only need P8 blocks <=13) into
        #      s_blocks 14/15, where no conv work is left to fill TensorE
        #      while ScalarE drains the last exps. ----
        attn_ps = {}

        def attn_pairs(uu, pairs):
            cc, t = uu // 4, uu % 4
            if uu not in attn_ps:
                pool = psc if uu < 2 or uu % 2 == 0 else ps
                attn_ps[uu] = pool.tile([128, 1024], f32,
                                        tag=("psc" if pool is psc else "ps"),
                                        name="p_at")
            p = attn_ps[uu]
            for j in pairs:
                for su in range(2):
                    o = t * 1024 + su * 512
                    nc.tensor.matmul(
                        p[:, su * 512 : (su + 1) * 512],
                        vT8[:, 2 * j : 2 * j + 2, cc * 128 : (cc + 1) * 128],
                        P8[:, 2 * j : 2 * j + 2, o : o + 512],
                        start=(j == 0), stop=(j == NB // 2 - 1),
                        perf_mode=mybir.MatmulPerfMode.DoubleRow,
                    )

        def attn_close(uu):
            cc, t = uu // 4, uu % 4
            p = attn_ps.pop(uu)
            st = big.tile([128, 1024], bf16, tag=("h2" if uu % 2 else "x3c"), name="st_a")
            nc.vector.tensor_scalar(
                st[:, 0:512], p[:, 0:512], u2G[:, cc : cc + 1], float(2.0 ** -G),
                op0=mybir.AluOpType.add, op1=mybir.AluOpType.mult,
            )
            nc.scalar.activation(
                st[:, 512:1024], p[:, 512:1024], AF.Identity,
                bias=u1[:, cc : cc + 1], scale=float(2.0 ** -G),
            )
            (nc.sync if uu % 2 else nc.gpsimd).dma_start(
                out=oa_d[cc * 128 : (cc + 1) * 128, t * 1024 : t * 1024 + 512],
                in_=st[:, 0:512],
            )
            (nc.scalar if uu % 2 else nc.sync).dma_start(
                out=oa_d[cc * 128 : (cc + 1) * 128, t * 1024 + 512 : (t + 1) * 1024],
                in_=st[:, 512:1024],
            )

        s_block(12)
        conv2_piece(1, 18, wb2, 0)
        s_block(13)
        conv2_piece(1, 26, wb2, 1)
        s_block(14, weave=[
            lambda: attn_pairs(0, [0, 1]), lambda: attn_pairs(0, [2, 3]),
            lambda: attn_pairs(0, [4, 5]), lambda: attn_pairs(0, [6]),
        ])
        s_block(15, weave=[
            lambda: attn_pairs(1, [0, 1]), lambda: attn_pairs(1, [2, 3]),
            lambda: attn_pairs(1, [4, 5]), lambda: attn_pairs(1, [6]),
        ])
        attn_pairs(0, [7])
        attn_close(0)
        attn_pairs(1, [7])
        attn_close(1)
        # remaining groups run pairwise with the j-loop outer (the pair shares
        # cc so one LDWEIGHTS serves both); closes are software-pipelined
        # behind the next pair's first matmuls to hide evac latency.
        for j in range(NB // 2):
            attn_pairs(2, [j])
            attn_pairs(3, [j])
        for j in range(2):
            attn_pairs(4, [j])
            attn_pairs(5, [j])
        attn_close(2)
        attn_close(3)
        for j in range(2, NB // 2):
            attn_pairs(4, [j])
            attn_pairs(5, [j])
        for j in range(2):
            attn_pairs(6, [j])
            attn_pairs(7, [j])
        attn_close(4)
        attn_close(5)
        for j in range(2, NB // 2):
            attn_pairs(6, [j])
            attn_pairs(7, [j])
        attn_close(6)
        attn_close(7)

    nc.compile()
    return nc


def _get_nc():
    if "nc" not in _CACHE:
        _CACHE["nc"] = _build_nc()
    return _CACHE["nc"]


def _make_in_maps(x, w1, b1, w2, b2, w3, b3, wb1, bb1, wb2, bb2,
                  wq, bq, wk, bk, wv, bv):
    bfc = lambda a: np.ascontiguousarray(np.asarray(a, np.float32).astype(ml_dtypes.bfloat16))
    f32c = lambda a: np.ascontiguousarray(np.asarray(a, np.float32))

    def qkv_t(w):  # [O, CI] -> lhsT/rhs chunks [128, 2, 256]
        return bfc(np.asarray(w, np.float32).T.reshape(2, 128, 256).transpose(1, 0, 2))

    def conv_t(wb):  # [O, I, 3, 3] -> [128 kip, 2 ki, 9 tap, 256 o]
        a = np.asarray(wb, np.float32).transpose(1, 0, 2, 3)  # [I, O, 3, 3]
        a = a.reshape(2, 128, 256, 9)                          # [ki, kip, o, tap]
        return bfc(a.transpose(1, 0, 3, 2))                    # [kip, ki, tap, o]

    def bias2(b):  # [256] -> [128, 2] (col cc = chunk cc)
        return f32c(np.asarray(b, np.float32).reshape(2, 128).T)

    wsb2 = np.zeros((128, 466), np.float32)
    wsb2[0:3, 0:64] = np.asarray(w1).T
    wsb2[3, 0:64] = np.asarray(b1)        # pairs with xq's ones row
    wsb2[0:64, 64:192] = np.asarray(w2).T
    wsb2[64, 64:192] = np.asarray(b2)     # pairs with h1's ones row
    wsb2[:, 192:448] = np.asarray(w3).T
    wsb2[0:64, 448] = np.asarray(b1)
    wsb2[:, 449] = np.asarray(b2)
    wsb2[:, 450:452] = bias2(b3)
    wsb2[:, 452:454] = bias2(bb1)
    wsb2[:, 454:456] = bias2(bb2)
    # cols 456/457 = per-core mtop/mbot, filled below
    wsb2[:, 458:460] = bias2(bq) * 64.0
    wsb2[:, 460:462] = bias2(bk) * 64.0
    wsb2[:, 462:464] = bias2(bq) * 256.0
    wsb2[:, 464:466] = bias2(bk) * 256.0
    wvb = np.zeros((128, 768), np.float32)
    wvb[:, 0:512] = qkv_t(wv).astype(np.float32).reshape(128, 512)
    wvb[0, 512:768] = np.asarray(bv)
    wqk8 = np.zeros((128, 1024), np.float32)
    wqk8[:, 0:512] = qkv_t(wq).astype(np.float32).reshape(128, 512) * 32.0
    wqk8[:, 512:1024] = qkv_t(wk).astype(np.float32).reshape(128, 512) * 32.0
    common = {
        "wvb": bfc(wvb),
        "wqk8": np.ascontiguousarray(wqk8.astype(ml_dtypes.float8_e4m3)),
        "wb1": conv_t(wb1),
        "wb2": conv_t(wb2),
    }

    xf = np.asarray(x, np.float32).reshape(B, 3, N)
    ones_row = np.ones((1, N), np.float32)
    in_maps = []
    for core in range(8):
        b, h = core // 2, core % 2
        xq = bfc(np.concatenate([np.roll(xf[b], -NH * h, axis=1), ones_row]))
        # conv window: global rows [32h-2, 32h+34), zero outside the image
        wc = wsb2.copy()
        wc[:, 456] = 0.0 if h == 0 else 1.0
        wc[:, 457] = 1.0 if h == 0 else 0.0
        in_maps.append(dict(
            common,
            xq=xq,
            wsb2=bfc(wc),
        ))
    return in_maps


def _gather(results, alpha, beta):
    a, bt = float(alpha), float(beta)
    out = np.empty((B, C, H, W), np.float32)
    for b in range(B):
        r0, r1 = results[2 * b], results[2 * b + 1]
        oa0 = np.asarray(r0["out_attn"], np.float32)
        oa1 = np.asarray(r1["out_attn"], np.float32)
        attn = oa0 + np.roll(oa1, NH, axis=1)
        conv = np.concatenate(
            [np.asarray(r0["out_conv"], np.float32).reshape(C, 32, W),
             np.asarray(r1["out_conv"], np.float32).reshape(C, 32, W)],
            axis=1,
        )
        out[b] = a * conv + bt * attn.reshape(C, H, W)
    return out


def _run(inputs, trace=False, **kw):
    from concourse import bass_utils

    nc = _get_nc()
    in_maps = _make_in_maps(
        inputs["x"], inputs["w1"], inputs["b1"], inputs["w2"], inputs["b2"],
        inputs["w3"], inputs["b3"], inputs["wb1"], inputs["bb1"],
        inputs["wb2"], inputs["bb2"], inputs["wq"], inputs["bq"],
        inputs["wk"], inputs["bk"], inputs["wv"], inputs["bv"],
    )
    res = bass_utils.run_bass_kernel_spmd(
        nc, in_maps, core_ids=list(range(8)), trace=trace, **kw
    )
    return _gather(res.results, inputs["alpha"], inputs["beta"]), res


def kernel(**inputs):
    out, _ = _run(inputs, trace=False)
    return out



# revision 56
# speedup vs baseline: 1.1941x; 1.1855x over previous
"""Trainium2 Bass kernel for nn_AttCM: 1x1-conv stem -> (two 3x3 convs) +
(single-head spatial attention), alpha/beta combined.

Sharding: 8 cores = 4 samples x 2 halves of the attention key axis (n).
Each core computes the full stem + q for its sample (cheap), its n-half of
S = k^T q with full softmax rows (softmax axis is m, fully local), a partial
attn_out = (v/l) @ exp(S) (host adds the two partials), and half of the 3x3
conv branch rows. No cross-core communication; the host applies
alpha*conv + beta*attn and the inverse of the per-core pixel roll.

SPMD trick: all 8 cores run one graph. Per-core behavior comes from data:
  - xq is the sample pixel-rolled by -2048*h so the core's k/v half is always
    columns [0, 2048) of its local x3; the attention output columns are rolled
    back on the host.
  - xc is a 36-row window of the sample (host zero-padded at image borders)
    so the conv branch always computes local output rows 2..33.
  - mtop/mbot (0.0 or 1.0 per core) zero the stem-of-zero padding rows that
    a true conv 'SAME' zero-pad requires.

Precision: conv matmuls are bf16 (fp32 PSUM accumulation). The whole
attention path runs fp8-e4m3 DoubleRow (K=256/instruction, ~1.5x TensorE):
q/k projections consume x3*8 against wq/wk*32; S = k^T q consumes q/k scaled
x64 (the x4096 compensates inside the ACT exp scale). The attention output
matmul is fp8 via mean-subtraction: the softmax here is near-uniform
(S in ~[-0.33, 0.30]), so attn = u + (v8 @ dev8)*2^-G where
u[c] = sum_n v[c,n]/4096 is computed exactly in bf16 (1-col matmuls, folded
into the psum evacuation as a per-partition bias) and only the deviation
dev = (exp(S)/l - 1/4096)*2^G goes through fp8e4 (G=16 centers it in e4m3;
plain fp8 softmax would flush the ~2.4e-4 rows to zero). b1/b2 ride the
stem matmuls via ones rows (xq row 3, h1 row 64) so the early evacuations
have no bias-DMA dependency. Outputs ship bf16. Measured rel_l2: 3.2e-3.

Schedule notes: TensorE is the global bottleneck (~152us busy at 2.4GHz).
The S loop is co-paced by ScalarE exp (1024-col chunks + READ_ACCUMULATOR),
so vT/u (s_blocks 0-1), the 3x3 conv pieces (blocks 1-13), and the first two
attention psum groups (j<=6 pairs, woven into blocks 14/15) fill TensorE.
PSUM runs as two independent 2-buf pools (4 banks each): S/stem chunks on
one, conv pieces + open attention accumulators on the other; stem chunks
alternate pools for 4-deep rotation. Remaining attention groups run pairwise
(shared-cc LDWEIGHTS) with closes software-pipelined. Stem evacuations split
512-col halves across ScalarE+VectorE (the stem is evac-latency-bound).
Input DMA: small/critical tensors (xq, stem weights+biases) load first on 3
queues; bulk weights kick later from the scalar queue to avoid descriptor
contention. Measured ~178us at full clock (~213us when the shared device
throttles to ~2.0GHz); baseline was ~212us full clock.
"""

import numpy as np
import ml_dtypes

_CACHE = {}

B, C, H, W = 4, 256, 64, 64
N = H * W            # 4096 pixels
NH = N // 2          # per-core attention key half
NB = 16              # n-blocks of 128 rows per core
G = 16               # 2^G boost of the softmax deviation before fp8e4 cast


def _build_nc():
    from contextlib import ExitStack

    import concourse.mybir as mybir
    import concourse.tile as tile
    from concourse import bacc

    f32 = mybir.dt.float32
    bf16 = mybir.dt.bfloat16
    f8 = mybir.dt.float8e4
    AF = mybir.ActivationFunctionType
    AX = mybir.AxisListType

    nc = bacc.Bacc("TRN2", target_bir_lowering=False, debug=False)

    def din(name, shape, dt=bf16):
        return nc.dram_tensor(name, shape, dt, kind="ExternalInput").ap()

    xq_d = din("xq", [4, N])  # row 3 = ones: b1 rides w1t row 3
    wsb2_d = din("wsb2", [128, 466])
    wvb_d = din("wvb", [128, 768])
    wqk8_d = din("wqk8", [128, 1024], f8)
    wb1_d = din("wb1", [128, 2, 9, 256])
    wb2_d = din("wb2", [128, 2, 9, 256])

    oa_d = nc.dram_tensor("out_attn", [C, N], bf16, kind="ExternalOutput").ap()
    oc_d = nc.dram_tensor("out_conv", [C, 32 * 64], bf16, kind="ExternalOutput").ap()

    with tile.TileContext(nc) as tc, ExitStack() as ctx:
        singles = ctx.enter_context(tc.tile_pool(name="singles", bufs=1))
        # two independent 2-buf psum pools (4 banks each): S-blocks/stem on
        # `ps`, conv pieces + early attn groups on `psc`, so attention
        # accumulators can stay open across s_blocks 14/15 without a
        # round-robin cycle against the S chunks.
        ps = ctx.enter_context(tc.tile_pool(name="ps", bufs=2, space="PSUM"))
        psc = ctx.enter_context(tc.tile_pool(name="psc", bufs=2, space="PSUM"))
        big = ctx.enter_context(tc.tile_pool(name="big", bufs=1))

        def load(d, shape, dt=bf16, tag=None):
            nm = d.tensor.name + "_sb"
            t = (singles.tile(shape, dt, tag=tag, name=nm) if tag
                 else singles.tile(shape, dt, name=nm))
            nc.sync.dma_start(out=t, in_=d)
            return t

        xq = big.tile([4, N], bf16, tag="x_in")
        nc.sync.dma_start(out=xq, in_=xq_d)  # first on sync queue: gates h1
        wsb2 = singles.tile([128, 466], bf16, name="wsb2")
        wvb = singles.tile([128, 768], bf16, name="wvb")
        wqk8 = singles.tile([128, 1024], f8, name="wqk8")
        # stem weights + biases split over the 3 DMA-capable queues: one queue
        # serializes 128 row-descriptors (~37 ns each), delaying the stem.
        # w1t (rows 0:3) ships first so h1 can start as soon as xq lands.
        nc.sync.dma_start(out=wsb2[0:4, :], in_=wsb2_d[0:4, :])
        nc.sync.dma_start(out=wsb2[4:44, :], in_=wsb2_d[4:44, :])
        nc.scalar.dma_start(out=wsb2[44:88, :], in_=wsb2_d[44:88, :])
        nc.gpsimd.dma_start(out=wsb2[88:128, :], in_=wsb2_d[88:128, :])
        # bulk weights (wqk8/wvb/wb1/wb2) kick from the scalar queue *after*
        # stem work starts: their ~500 descriptors otherwise contend with the
        # small early loads on the shared DMA channels and delay the biases.
        w1t = wsb2[0:4, 0:64]    # row 3 = b1 (pairs with xq's ones row)
        w2t = wsb2[0:65, 64:192]  # row 64 = b2 (pairs with h1's ones row)
        w3t = wsb2[:, 192:448]
        wvt = wvb[:, 0:512].rearrange("p (a b) -> p a b", a=2)
        bv = wvb[0:1, 512:768]
        # biases ship as bf16 inside wsb2 (a separate narrow f32 tensor costs
        # 128 DMA descriptors); DVE scalar operands must be f32, so convert
        # the 18 bias columns on-chip once.
        fsb = singles.tile([128, 18], f32, name="fsb32")
        nc.vector.tensor_copy(fsb, wsb2[:, 448:466])
        b1 = fsb[0:64, 0:1]
        b2 = fsb[:, 1:2]
        b3 = fsb[:, 2:4]
        bb1 = fsb[:, 4:6]
        bb2 = fsb[:, 6:8]
        mtop = fsb[:, 8:9]
        mbot = fsb[:, 9:10]
        bq64 = fsb[:, 10:12]
        bk64 = fsb[:, 12:14]
        bq256 = fsb[:, 14:16]
        bk256 = fsb[:, 16:18]
        wq8t = wqk8[:, 0:512].rearrange("p (a b) -> p a b", a=2)
        wk8t = wqk8[:, 512:1024].rearrange("p (a b) -> p a b", a=2)
        ones = singles.tile([1, 128], bf16)
        nc.vector.memset(ones, 1.0)
        ones128 = singles.tile([128, 1], bf16)
        nc.vector.memset(ones128, 1.0)
        lall = singles.tile([128, NB], f32)
        lg = singles.tile([128, NB], f32)
        rl = singles.tile([128, NB], f32)
        u2G = singles.tile([128, 2], f32)
        u1 = singles.tile([128, 2], f32)

        # ---- stem on the rolled full sample (feeds q, k, v) ----
        # evacuations run 512-col halves on ScalarE and VectorE in parallel:
        # the stem is a latency chain (h1 -> h2 -> x3), so evac latency, not
        # throughput, sets its span.
        def pstile(i, part=128, name="p_st"):
            # stem-phase chunks alternate between the two psum pools so the
            # evac-latency chain overlaps 4 tiles even with 2-buf pools
            pool, tg = (ps, "ps") if i % 2 == 0 else (psc, "psc")
            return pool.tile([part, 1024], f32, tag=tg, name=name)

        def relu_evac(dst, p, b, t):
            # b=None: bias already folded into the matmul via a ones row
            sh = t % 2
            vh = 1 - sh
            ssl = slice(sh * 512, sh * 512 + 512)
            vsl = slice(vh * 512, vh * 512 + 512)
            if b is None:
                nc.scalar.activation(dst[:, ssl], p[:, ssl], AF.Relu)
                nc.vector.tensor_scalar_max(dst[:, vsl], p[:, vsl], 0.0)
            else:
                nc.scalar.activation(dst[:, ssl], p[:, ssl], AF.Relu, bias=b)
                nc.vector.tensor_scalar(dst[:, vsl], p[:, vsl], b, 0.0,
                                        op0=mybir.AluOpType.add, op1=mybir.AluOpType.max)

        h1 = big.tile([65, N], bf16, tag="h1")
        nc.vector.memset(h1[64:65, :], 1.0)  # ones row: b2 rides w2t row 64
        for t in range(4):
            p = pstile(t, part=64, name="p_h1")
            for su in range(2):
                nc.tensor.matmul(
                    p[:, su * 512 : (su + 1) * 512], w1t,
                    xq[:, t * 1024 + su * 512 : t * 1024 + (su + 1) * 512],
                    start=True, stop=True,
                )
            relu_evac(h1[0:64, t * 1024 : (t + 1) * 1024], p, None, t)
        nc.scalar.dma_start(out=wqk8, in_=wqk8_d)
        nc.scalar.dma_start(out=wvb, in_=wvb_d)
        h2 = big.tile([128, N], bf16, tag="h2")
        for t in range(4):
            p = pstile(t + 1, name="p_h2")
            for su in range(2):
                nc.tensor.matmul(
                    p[:, su * 512 : (su + 1) * 512], w2t,
                    h1[:, t * 1024 + su * 512 : t * 1024 + (su + 1) * 512],
                    start=True, stop=True,
                )
            relu_evac(h2[:, t * 1024 : (t + 1) * 1024], p, None, t)
        # x3 runs t-major so both cc chunks of an m-range finish together and
        # the q projection (which needs both ki of a chunk) can start early;
        # the fp8 x38 chunk-casts alternate ScalarE/VectorE right behind.
        x3q = big.tile([128, 2, N], bf16, tag="x3q")
        x38 = big.tile([128, 2, N], f8, tag="h1")  # reuses dead h1 space
        for t in range(4):
            for cc in range(2):
                p = pstile(t * 2 + cc, name="p_x3q")
                for su in range(2):
                    nc.tensor.matmul(
                        p[:, su * 512 : (su + 1) * 512],
                        w3t[:, cc * 128 : (cc + 1) * 128],
                        h2[:, t * 1024 + su * 512 : t * 1024 + (su + 1) * 512],
                        start=True, stop=True,
                    )
                csl = slice(t * 1024, (t + 1) * 1024)
                relu_evac(x3q[:, cc, csl], p, b3[:, cc : cc + 1], t + cc)
                if (t + cc) % 2:
                    nc.scalar.activation(x38[:, cc, csl], x3q[:, cc, csl],
                                         AF.Identity, scale=8.0)
                else:
                    nc.vector.tensor_scalar_mul(x38[:, cc, csl], x3q[:, cc, csl], 8.0)

        # ---- k, q (t-major: both cc of a chunk land together so the S loop
        #      can start after chunk 0), fp8 DoubleRow projections from x38 ----
        k_ = big.tile([128, 2, NH], f8, tag="k")
        for t in range(2):
            for cc in range(2):
                p = pstile(t * 2 + cc, name="p_k")
                for su in range(2):
                    nc.tensor.matmul(
                        p[:, su * 512 : (su + 1) * 512],
                        wk8t[:, :, cc * 128 : (cc + 1) * 128],
                        x38[:, :, t * 1024 + su * 512 : t * 1024 + (su + 1) * 512],
                        start=True, stop=True,
                        perf_mode=mybir.MatmulPerfMode.DoubleRow,
                    )
                sh = (t + cc) % 2
                vh = 1 - sh
                nc.scalar.activation(
                    k_[:, cc, t * 1024 + sh * 512 : t * 1024 + sh * 512 + 512],
                    p[:, sh * 512 : sh * 512 + 512], AF.Identity,
                    bias=bk64[:, cc : cc + 1], scale=0.25,
                )
                nc.vector.tensor_scalar(
                    k_[:, cc, t * 1024 + vh * 512 : t * 1024 + vh * 512 + 512],
                    p[:, vh * 512 : vh * 512 + 512], bk256[:, cc : cc + 1], 0.25,
                    op0=mybir.AluOpType.add, op1=mybir.AluOpType.mult,
                )
        q = big.tile([128, 2, N], f8, tag="q")
        for t in range(4):
            for cc in range(2):
                p = pstile(t * 2 + cc, name="p_q")
                for su in range(2):
                    nc.tensor.matmul(
                        p[:, su * 512 : (su + 1) * 512],
                        wq8t[:, :, cc * 128 : (cc + 1) * 128],
                        x38[:, :, t * 1024 + su * 512 : t * 1024 + (su + 1) * 512],
                        start=True, stop=True,
                        perf_mode=mybir.MatmulPerfMode.DoubleRow,
                    )
                sh = (t + cc) % 2
                vh = 1 - sh
                nc.scalar.activation(
                    q[:, cc, t * 1024 + sh * 512 : t * 1024 + sh * 512 + 512],
                    p[:, sh * 512 : sh * 512 + 512], AF.Identity,
                    bias=bq64[:, cc : cc + 1], scale=0.25,
                )
                nc.vector.tensor_scalar(
                    q[:, cc, t * 1024 + vh * 512 : t * 1024 + vh * 512 + 512],
                    p[:, vh * 512 : vh * 512 + 512], bq256[:, cc : cc + 1], 0.25,
                    op0=mybir.AluOpType.add, op1=mybir.AluOpType.mult,
                )

        # vT[n, c] = sum_ci x3[ci, n] WvT[ci, c] + bv[c]  (bias via K=1 matmul)
        # issued later, woven into s_blocks 0/1 to fill TensorE while the
        # Scalar/Vector queues drain the stem evac backlog.
        vT = big.tile([128, NB, 256], bf16, tag="vT")
        vT8 = big.tile([128, NB, 256], f8, tag="vT8")

        def vt_group(g):
            p = pstile(g, name="p_vT")
            for j in range(4):
                nb = g * 4 + j
                nsl = slice(nb * 128, (nb + 1) * 128)
                o = slice(j * 256, (j + 1) * 256)
                nc.tensor.matmul(p[:, o], x3q[:, 0, nsl], wvt[:, 0, :], start=True, stop=False)
                nc.tensor.matmul(p[:, o], x3q[:, 1, nsl], wvt[:, 1, :], start=False, stop=False)
                nc.tensor.matmul(p[:, o], ones, bv, start=False, stop=True)
            nc.vector.tensor_copy(vT[:, g * 4 : (g + 1) * 4, :], p)
            nc.vector.tensor_copy(vT8[:, g * 4 : (g + 1) * 4, :], vT[:, g * 4 : (g + 1) * 4, :])

        # mean term u[c] = (sum_n v[c, n]) / 4096, exact in bf16; folded into
        # the attention-psum evacuation. u2G = u*2^G for the VectorE evac
        # (psum + u2G)*2^-G; u1 = u for the ScalarE evac 2^-G*psum + u.
        def u_mms():
            p_u = pstile(1, name="p_u")
            for cc in range(2):
                for nb in range(NB):
                    nc.tensor.matmul(
                        p_u[:, cc : cc + 1], vT[:, nb, cc * 128 : (cc + 1) * 128],
                        ones128, start=(nb == 0), stop=(nb == NB - 1),
                    )
            nc.scalar.activation(u2G, p_u[:, 0:2], AF.Identity, scale=float(2.0 ** G) / 4096.0)
            nc.scalar.activation(u1, p_u[:, 0:2], AF.Identity, scale=1.0 / 4096.0)

        # ---- conv input: x3c is x3q in the rolled frame — local window row
        #      j (0..35) = rolled row (j-2) mod 64; the per-core mtop/mbot
        #      masks zero the rows that are conv 'SAME' padding (the wrap rows
        #      land exactly where the masks already zero or keep correctly).
        #      cc=0 builds on the otherwise-idle GpSimd, cc=1 on Vector/Scalar.
        x3c = big.tile([128, 2, 36, 66], bf16, tag="x3c")
        # only cols 0 and 65 are never written below — zero just the borders
        nc.gpsimd.memset(x3c[:, :, :, 0:1], 0.0)
        nc.gpsimd.memset(x3c[:, :, :, 65:66], 0.0)
        for cc in range(2):
            nc.vector.tensor_copy(
                x3c[:, cc, 2:36, 1:65],
                x3q[:, cc, 0 : 34 * 64].rearrange("p (a b) -> p a b", a=34),
            )
            nc.vector.tensor_copy(
                x3c[:, cc, 0:2, 1:65],
                x3q[:, cc, 62 * 64 : 64 * 64].rearrange("p (a b) -> p a b", a=2),
            )
        # zero the stem-of-zero border rows (true 'SAME' pad is zero in x3)
        for cc in range(2):
            nc.vector.tensor_scalar_mul(x3c[:, cc, 0:2, :], x3c[:, cc, 0:2, :], mtop)
            nc.vector.tensor_scalar_mul(x3c[:, cc, 34:36, :], x3c[:, cc, 34:36, :], mbot)

        wb1 = singles.tile([128, 2, 9, 256], bf16, tag="wb", name="wb1_sb")
        nc.scalar.dma_start(out=wb1, in_=wb1_d)
        wb2 = singles.tile([128, 2, 9, 256], bf16, tag="wb2", name="wb2_sb")
        nc.scalar.dma_start(out=wb2, in_=wb2_d)
        y1p0 = big.tile([128, 34, 66], bf16, tag="h1")
        y1p1 = big.tile([128, 34, 66], bf16, tag="x_in")
        y1p_ = lambda ki: y1p0 if ki == 0 else y1p1
        # conv1 evacs fill cols 1:65 of every row; zero only the borders
        for yp in (y1p0, y1p1):
            nc.gpsimd.memset(yp[:, :, 0:1], 0.0)
            nc.gpsimd.memset(yp[:, :, 65:66], 0.0)

        # ---- S-loop / conv pieces (interleaved below) ----
        # P8[n, m] = (exp(S)/l - 1/4096) * 2^G in fp8e4: the softmax here is
        # near-uniform, so only the *deviation* from the uniform 1/4096 row
        # goes through fp8 (the exact mean term u is added at evacuation).
        P8 = big.tile([128, NB, N], f8, tag="P8")

        def s_block(nb, weave=()):
            nsl = slice(nb * 128, (nb + 1) * 128)
            lp = singles.tile([128, 4], f32, tag="lp", bufs=4, name="lp")
            pst = big.tile([128, N], bf16, tag="Pst", bufs=3, name="Pst")
            for t in range(4):
                p = ps.tile([128, 1024], f32, tag="ps", name="p_s")
                for su in range(2):
                    o = t * 1024 + su * 512
                    nc.tensor.matmul(
                        p[:, su * 512 : (su + 1) * 512],
                        k_[:, :, nsl], q[:, :, o : o + 512],
                        start=True, stop=True,
                        perf_mode=mybir.MatmulPerfMode.DoubleRow,
                    )
                if t < len(weave) and weave[t] is not None:
                    weave[t]()
                nc.scalar.activation(
                    pst[:, t * 1024 : (t + 1) * 1024], p, AF.Exp,
                    scale=1.0 / 4096.0, accum_out=lp[:, t : t + 1],
                )
            nc.vector.reduce_sum(out=lall[:, nb : nb + 1], in_=lp, axis=AX.X)
            nc.vector.tensor_scalar_mul(lg[:, nb : nb + 1], lall[:, nb : nb + 1],
                                        float(2.0 ** -G))
            nc.vector.reciprocal(rl[:, nb : nb + 1], lg[:, nb : nb + 1])
            # the last two blocks cast in m-halves: the j=7 attention matmuls
            # of the early groups (t<2) unlock after the first half
            halves = 2 if nb >= NB - 2 else 1
            for hh in range(halves):
                msl = slice(hh * (N // halves), (hh + 1) * (N // halves))
                nc.vector.tensor_scalar(
                    P8[:, nb, msl], pst[:, msl], rl[:, nb : nb + 1],
                    -(2.0 ** G) / 4096.0,
                    op0=mybir.AluOpType.mult, op1=mybir.AluOpType.add,
                )

        def conv1_piece(cc, y1row0, nr=8):
            """nr y1-rows in one psum bank."""
            w = nr * 64
            p = psc.tile([128, 1024], f32, tag="psc", name="p_c1")
            for kt in range(18):
                ki, tap = kt // 9, kt % 9
                dh, dw = tap // 3, tap % 3
                nc.tensor.matmul(
                    p[:, 0:w],
                    wb1[:, ki, tap, cc * 128 : (cc + 1) * 128],
                    x3c[:, ki, y1row0 - 1 + dh : y1row0 - 1 + dh + nr, dw : dw + 64],
                    start=(kt == 0), stop=(kt == 17),
                )
            nc.vector.tensor_scalar(
                y1p_(cc)[:, y1row0 - 1 : y1row0 - 1 + nr, 1:65], p[:, 0:w],
                bb1[:, cc : cc + 1], 0.0,
                op0=mybir.AluOpType.add, op1=mybir.AluOpType.max,
            )

        def conv2_piece(cc, orow0, wb2, sti, nr=8):
            w = nr * 64
            p = psc.tile([128, 1024], f32, tag="psc", name="p_c2")
            for kt in range(18):
                ki, tap = kt // 9, kt % 9
                dh, dw = tap // 3, tap % 3
                nc.tensor.matmul(
                    p[:, 0:w],
                    wb2[:, ki, tap, cc * 128 : (cc + 1) * 128],
                    y1p_(ki)[:, orow0 - 2 + dh : orow0 - 2 + dh + nr, dw : dw + 64],
                    start=(kt == 0), stop=(kt == 17),
                )
            st = big.tile([128, 1024], bf16, tag=("h2" if sti else "x3c"), name="st_c")
            nc.vector.tensor_scalar_add(st[:, 0:w], p[:, 0:w], bb2[:, cc : cc + 1])
            (nc.sync if sti else nc.gpsimd).dma_start(
                out=oc_d[cc * 128 : (cc + 1) * 128, (orow0 - 2) * 64 : (orow0 - 2) * 64 + w],
                in_=st[:, 0:w],
            )

        # ---- interleave: S blocks are ScalarE(exp)-paced; vT/u then conv
        #      groups keep TensorE busy meanwhile ----
        s_block(0, weave=[lambda: vt_group(0), lambda: vt_group(1),
                          lambda: vt_group(2), lambda: vt_group(3)])
        s_block(1, weave=[u_mms, None, lambda: conv1_piece(0, 1)])
        conv1_piece(0, 9)
        s_block(2)
        conv1_piece(0, 17)
        conv1_piece(0, 25)
        s_block(3)
        conv1_piece(1, 1)
        conv1_piece(1, 9)
        s_block(4)
        conv1_piece(1, 17)
        conv1_piece(1, 25)
        s_block(5)
        conv1_piece(0, 33, nr=2)
        conv1_piece(1, 33, nr=2)
        for cc in range(2):
            nc.vector.tensor_scalar_mul(y1p_(cc)[:, 0, :], y1p_(cc)[:, 0, :], mtop)
            nc.vector.tensor_scalar_mul(y1p_(cc)[:, 33, :], y1p_(cc)[:, 33, :], mbot)
        s_block(6)
        conv2_piece(0, 2, wb2, 0)
        s_block(7)
        conv2_piece(0, 10, wb2, 1)
        s_block(8)
        conv2_piece(0, 18, wb2, 0)
        s_block(9)
        conv2_piece(0, 26, wb2, 1)
        s_block(10)
        conv2_piece(1, 2, wb2, 0)
        s_block(11)
        conv2_piece(1, 10, wb2, 1)
        # ---- attn_out partial = v @ P8 * 2^-G + u; fp8 DoubleRow (K=256 per
        #      instruction over nb pairs). Groups 0/1 open early and weave
        #      their j<=6 pairs (which only need P8 blocks <=13) into
        #      s_blocks 14/15, where no conv work is left to fill TensorE
        #      while ScalarE drains the last exps. ----
        attn_ps = {}

        def attn_pairs(uu, pairs):
            cc, t = uu // 4, uu % 4
            if uu not in attn_ps:
                pool = psc if uu < 2 or uu % 2 == 0 else ps
                attn_ps[uu] = pool.tile([128, 1024], f32,
                                        tag=("psc" if pool is psc else "ps"),
                                        name="p_at")
            p = attn_ps[uu]
            for j in pairs:
                for su in range(2):
                    o = t * 1024 + su * 512
                    nc.tensor.matmul(
                        p[:, su * 512 : (su + 1) * 512],
                        vT8[:, 2 * j : 2 * j + 2, cc * 128 : (cc + 1) * 128],
                        P8[:, 2 * j : 2 * j + 2, o : o + 512],
                        start=(j == 0), stop=(j == NB // 2 - 1),
                        perf_mode=mybir.MatmulPerfMode.DoubleRow,
                    )

        def attn_close(uu):
            cc, t = uu // 4, uu % 4
            p = attn_ps.pop(uu)
            st = big.tile([128, 1024], bf16, tag=("h2" if uu % 2 else "x3c"), name="st_a")
            nc.vector.tensor_scalar(
                st[:, 0:512], p[:, 0:512], u2G[:, cc : cc + 1], float(2.0 ** -G),
                op0=mybir.AluOpType.add, op1=mybir.AluOpType.mult,
            )
            nc.scalar.activation(
                st[:, 512:1024], p[:, 512:1024], AF.Identity,
                bias=u1[:, cc : cc + 1], scale=float(2.0 ** -G),
            )
            (nc.sync if uu % 2 else nc.gpsimd).dma_start(
                out=oa_d[cc * 128 : (cc + 1) * 128, t * 1024 : t * 1024 + 512],
                in_=st[:, 0:512],
            )
            (nc.scalar if uu % 2 else nc.sync).dma_start(
                out=oa_d[cc * 128 : (cc + 1) * 128, t * 1024 + 512 : (t + 1) * 1024],
                in_=st[:, 512:1024],
            )

        s_block(12)
        conv2_piece(1, 18, wb2, 0)
        s_block(13)
        conv2_piece(1, 26, wb2, 1)
        s_block(14, weave=[
            lambda: attn_pairs(0, [0, 1]), lambda: attn_pairs(0, [2, 3]),
            lambda: attn_pairs(0, [4, 5]), lambda: attn_pairs(0, [6]),
        ])
        s_block(15, weave=[
            lambda: attn_pairs(1, [0, 1]), lambda: attn_pairs(1, [2, 3]),
            lambda: attn_pairs(1, [4, 5]), lambda: attn_pairs(1, [6]),
        ])
        attn_pairs(0, [7])
        attn_close(0)
        attn_pairs(1, [7])
        attn_close(1)
        # remaining groups run pairwise with the j-loop outer (the pair shares
        # cc so one LDWEIGHTS serves both); closes are software-pipelined
        # behind the next pair's first matmuls to hide evac latency.
        for j in range(NB // 2):
            attn_pairs(2, [j])
            attn_pairs(3, [j])
        for j in range(2):
            attn_pairs(4, [j])
            attn_pairs(5, [j])
        attn_close(2)
        attn_close(3)
        for j in range(2, NB // 2):
            attn_pairs(4, [j])
            attn_pairs(5, [j])
        for j in range(2):
            attn_pairs(6, [j])
            attn_pairs(7, [j])
        attn_close(4)
        attn_close(5)
        for j in range(2, NB // 2):
            attn_pairs(6, [j])
            attn_pairs(7, [j])
        attn_close(6)
        attn_close(7)

    nc.compile()
    return nc


def _get_nc():
    if "nc" not in _CACHE:
        _CACHE["nc"] = _build_nc()
    return _CACHE["nc"]


def _make_in_maps(x, w1, b1, w2, b2, w3, b3, wb1, bb1, wb2, bb2,
                  wq, bq, wk, bk, wv, bv):
    bfc = lambda a: np.ascontiguousarray(np.asarray(a, np.float32).astype(ml_dtypes.bfloat16))
    f32c = lambda a: np.ascontiguousarray(np.asarray(a, np.float32))

    def qkv_t(w):  # [O, CI] -> lhsT/rhs chunks [128, 2, 256]
        return bfc(np.asarray(w, np.float32).T.reshape(2, 128, 256).transpose(1, 0, 2))

    def conv_t(wb):  # [O, I, 3, 3] -> [128 kip, 2 ki, 9 tap, 256 o]
        a = np.asarray(wb, np.float32).transpose(1, 0, 2, 3)  # [I, O, 3, 3]
        a = a.reshape(2, 128, 256, 9)                          # [ki, kip, o, tap]
        return bfc(a.transpose(1, 0, 3, 2))                    # [kip, ki, tap, o]

    def bias2(b):  # [256] -> [128, 2] (col cc = chunk cc)
        return f32c(np.asarray(b, np.float32).reshape(2, 128).T)

    wsb2 = np.zeros((128, 466), np.float32)
    wsb2[0:3, 0:64] = np.asarray(w1).T
    wsb2[3, 0:64] = np.asarray(b1)        # pairs with xq's ones row
    wsb2[0:64, 64:192] = np.asarray(w2).T
    wsb2[64, 64:192] = np.asarray(b2)     # pairs with h1's ones row
    wsb2[:, 192:448] = np.asarray(w3).T
    wsb2[0:64, 448] = np.asarray(b1)
    wsb2[:, 449] = np.asarray(b2)
    wsb2[:, 450:452] = bias2(b3)
    wsb2[:, 452:454] = bias2(bb1)
    wsb2[:, 454:456] = bias2(bb2)
    # cols 456/457 = per-core mtop/mbot, filled below
    wsb2[:, 458:460] = bias2(bq) * 64.0
    wsb2[:, 460:462] = bias2(bk) * 64.0
    wsb2[:, 462:464] = bias2(bq) * 256.0
    wsb2[:, 464:466] = bias2(bk) * 256.0
    wvb = np.zeros((128, 768), np.float32)
    wvb[:, 0:512] = qkv_t(wv).astype(np.float32).reshape(128, 512)
    wvb[0, 512:768] = np.asarray(bv)
    wqk8 = np.zeros((128, 1024), np.float32)
    wqk8[:, 0:512] = qkv_t(wq).astype(np.float32).reshape(128, 512) * 32.0
    wqk8[:, 512:1024] = qkv_t(wk).astype(np.float32).reshape(128, 512) * 32.0
    common = {
        "wvb": bfc(wvb),
        "wqk8": np.ascontiguousarray(wqk8.astype(ml_dtypes.float8_e4m3)),
        "wb1": conv_t(wb1),
        "wb2": conv_t(wb2),
    }

    xf = np.asarray(x, np.float32).reshape(B, 3, N)
    ones_row = np.ones((1, N), np.float32)
    in_maps = []
    for core in range(8):
        b, h = core // 2, core % 2
        xq = bfc(np.concatenate([np.roll(xf[b], -NH * h, axis=1), ones_row]))
        # conv window: global rows [32h-2, 32h+34), zero outside the image
        wc = wsb2.copy()
        wc[:, 456] = 0.0 if h == 0 else 1.0
        wc[:, 457] = 1.0 if h == 0 else 0.0
        in_maps.append(dict(
            common,
            xq=xq,
            wsb2=bfc(wc),
        ))
    return in_maps


def _gather(results, alpha, beta):
    a, bt = float(alpha), float(beta)
    out = np.empty((B, C, H, W), np.float32)
    for b in range(B):
        r0, r1 = results[2 * b], results[2 * b + 1]
        oa0 = np.asarray(r0["out_attn"], np.float32)
        oa1 = np.asarray(r1["out_attn"], np.float32)
        attn = oa0 + np.roll(oa1, NH, axis=1)
        conv = np.concatenate(
            [np.asarray(r0["out_conv"], np.float32).reshape(C, 32, W),
             np.asarray(r1["out_conv"], np.float32).reshape(C, 32, W)],
            axis=1,
        )
        out[b] = a * conv + bt * attn.reshape(C, H, W)
    return out


def _run(inputs, trace=False, **kw):
    from concourse import bass_utils

    nc = _get_nc()
    in_maps = _make_in_maps(
        inputs["x"], inputs["w1"], inputs["b1"], inputs["w2"], inputs["b2"],
        inputs["w3"], inputs["b3"], inputs["wb1"], inputs["bb1"],
        inputs["wb2"], inputs["bb2"], inputs["wq"], inputs["bq"],
        inputs["wk"], inputs["bk"], inputs["wv"], inputs["bv"],
    )
    res = bass_utils.run_bass_kernel_spmd(
        nc, in_maps, core_ids=list(range(8)), trace=trace, **kw
    )
    return _gather(res.results, inputs["alpha"], inputs["beta"]), res


def kernel(**inputs):
    out, _ = _run(inputs, trace=False)
    return out

